# revision 1
# baseline (speedup 1.0000x reference)
"""Trainium2 Bass kernel for MultiHeadAttention (B=2, S=4096, D=512, H=8).

Sharding: 16 (batch, head) units across 8 cores -> each core owns one batch
and a contiguous pair of heads (2 heads x 64 depth = 128 columns of the
QKV projections, 128 rows of the output projection).

Key ideas:
  * Mask compression on host: keys with mask==1 receive -1e9 before softmax,
    so their probability is exactly 0 in fp32. We drop those keys entirely
    (gather unmasked rows of x2), roughly halving scores/softmax/AV work.
    Dropped-key handling is exact, not approximate.
  * Everything on device runs out of a transposed activation layout:
      Q_T, K_T: [128(=2 heads x 64 depth), S]  (from x1^T / x2c^T inputs)
    scores for one key-tile land as [128 keys, 1024(=2 heads x 512 queries)]
    in PSUM, and a single ScalarE activation does exp(scores/8) PSUM->SBUF.
    The key-padding mask rides along as an extra column of V, which makes
    the softmax denominator fall out of the same PE accumulation as A@V.
  * Normalization: reciprocal of the denominator row, broadcast across
    partitions with a K=1 matmul, one VectorE multiply per head; it is
    deferred into the next chunk's score loop so it overlaps.
  * K/V projection work for key-chunks 1.. streams inside chunk 0's score
    loop so the serialized input DMAs hide behind compute.
  * All matmul operands are float32r (same 4-byte layout as fp32; the PE's
    fast single-pass fp32 mode). Walrus requires every producer of an f32r
    matmul operand to emit f32r itself, hence the f32r tile dtypes.
  * Host sums the 4 per-core partial outputs of each batch (head groups are
    disjoint in Wo rows, so partials just add; bo added on host).

Measured (fixed seed inputs): max relative error 3.9e-04 vs the fp32
reference (f32r rounding); cost-model exec time ~171 us per core.  Non-zero
q/k/v biases or an all-masked batch fall back to a numpy reference (those
inputs cannot occur with the problem's setup_inputs).
"""

import numpy as np

B, S, D, H = 2, 4096, 512, 8
DH = 64  # depth per head
NCORES = 8

_RUNTIMES = {}


def _build_program(skc: int, reps: int = 1):
    """Build the per-core Bass program. skc = padded compressed key count."""
    import concourse.bacc as bacc
    import concourse.mybir as mybir
    from concourse.masks import make_identity
    from concourse.tile import TileContext

    f32 = mybir.dt.float32
    f32r = mybir.dt.float32r
    EXP = mybir.ActivationFunctionType.Exp
    CPY = mybir.ActivationFunctionType.Copy
    r = lambda ap: ap.bitcast(mybir.dt.float32r)  # fast fp32 matmul mode

    NT = skc // 128  # key tiles
    NQC = S // 512  # query chunks (512 wide)
    NKC = (skc + 511) // 512  # key chunks for the K/V projections

    nc = bacc.Bacc("TRN2", target_bir_lowering=False, debug=False, num_devices=NCORES)

    x1t = nc.dram_tensor("x1t", [D, S], f32r, kind="ExternalInput")
    x2ct = nc.dram_tensor("x2ct", [D, skc], f32r, kind="ExternalInput")
    maskf = nc.dram_tensor("maskf", [128, NT], f32, kind="ExternalInput")
    wq = nc.dram_tensor("wq", [D, 128], f32r, kind="ExternalInput")
    wk = nc.dram_tensor("wk", [D, 128], f32r, kind="ExternalInput")
    wv = nc.dram_tensor("wv", [D, 128], f32r, kind="ExternalInput")
    wo2 = nc.dram_tensor("wo2", [64, 1024], f32r, kind="ExternalInput")
    out = nc.dram_tensor("out", [S, D], f32, kind="ExternalOutput")

    with nc.allow_low_precision(
        reason="float32r tiles hold full-fp32 data; matmuls accumulate in fp32 PSUM"
    ), TileContext(nc) as tc:
        with (
            tc.tile_pool(name="consts", bufs=1) as consts,
            tc.tile_pool(name="bigsb", bufs=1) as bigsb,
            tc.tile_pool(name="xstream", bufs=3) as xstream,
            # pexp buffer count is SBUF-budget-bound and shrinks as the
            # (data-dependent) compressed key length grows
            tc.tile_pool(
                name="pexp",
                bufs=(12 if skc <= 2048 else 10 if skc <= 2432 else 6),
            ) as pexp,
            tc.tile_pool(name="work", bufs=3) as work,
            tc.tile_pool(name="ps_big", bufs=2, space="PSUM") as ps_big,
            tc.tile_pool(name="ps_oacc", bufs=2, space="PSUM") as ps_oacc,
            tc.tile_pool(name="ps_misc", bufs=2, space="PSUM") as ps_misc,
        ):
            # ---- constants / persistent buffers (DMA issue order matters:
            # the DMA device drains them in order) ----
            # x1 chunk 0 first, split per k-tile so the first Q matmul can
            # start after only a quarter of the transfer
            x1r = x1t.rearrange("(t p) s -> p t s", p=128)
            wq_sb = consts.tile([128, 4, 128], f32r)
            nc.sync.dma_start(out=wq_sb, in_=wq.rearrange("(t p) m -> p t m", p=128))
            x1c0 = xstream.tile([128, 4, 512], f32r, tag="xs")
            for kt in range(4):
                nc.sync.dma_start(
                    out=x1c0[:, kt, :], in_=x1r[:, kt, 0:512]
                )
            wk_sb = consts.tile([128, 4, 128], f32r)
            nc.sync.dma_start(out=wk_sb, in_=wk.rearrange("(t p) m -> p t m", p=128))
            x2all = bigsb.tile([128, 4, skc], f32r)
            x2r = x2ct.rearrange("(t p) s -> p t s", p=128)
            c0w = min(512, skc)
            c0a = min(128, c0w)  # first key-tile lands fast -> early first score
            nc.sync.dma_start(out=x2all[:, :, 0:c0a], in_=x2r[:, :, 0:c0a])
            wv_sb = consts.tile([128, 4, 128], f32r)
            nc.sync.dma_start(out=wv_sb, in_=wv.rearrange("(t p) m -> p t m", p=128))
            maskf_sb = consts.tile([128, NT], f32)
            nc.sync.dma_start(out=maskf_sb, in_=maskf[:, :])
            if c0w > c0a:
                nc.sync.dma_start(
                    out=x2all[:, :, c0a:c0w], in_=x2r[:, :, c0a:c0w]
                )
            for c in range(1, NKC):
                cw = min(512, skc - c * 512)
                nc.sync.dma_start(
                    out=x2all[:, :, c * 512 : c * 512 + cw],
                    in_=x2r[:, :, c * 512 : c * 512 + cw],
                )
            wo2_sb = consts.tile([64, 1024], f32r)
            nc.sync.dma_start(out=wo2_sb, in_=wo2[:, :])

            ones_f32 = consts.tile([65, 128], f32)
            nc.vector.memset(ones_f32, 1.0)
            ones65 = consts.tile([65, 128], f32r)
            nc.vector.tensor_copy(ones65, ones_f32)
            ident = consts.tile([128, 128], f32)
            make_identity(nc, ident)

            # ---- persistent activations ----
            q_t = bigsb.tile([128, S], f32r)
            k_t = bigsb.tile([128, skc], f32r)
            vaug = bigsb.tile([128, NT * 130], f32r)
            o_n0 = bigsb.tile([64, S], f32r)
            o_n1 = bigsb.tile([64, S], f32r)

            for _rep in range(reps):

                def emit_kv(c, lo=0, hi=None):
                    """K_T projection + V_T projection + V transpose + V_aug
                    assembly for key-chunk c, columns [lo, hi) of the chunk."""
                    cw = min(512, skc - c * 512) if hi is None else hi
                    ks = slice(c * 512 + lo, c * 512 + cw)
                    cw = cw - lo
                    psk = ps_misc.tile([128, 512], f32, tag="misc", name="psk")
                    for kt in range(4):
                        nc.tensor.matmul(
                            psk[:, :cw],
                            r(wk_sb[:, kt, :]) if cw >= 256 else wk_sb[:, kt, :],
                            r(x2all[:, kt, ks]) if cw >= 256 else x2all[:, kt, ks],
                            start=(kt == 0),
                            stop=(kt == 3),
                        )
                    nc.vector.tensor_copy(k_t[:, ks], psk[:, :cw])
                    psvt = ps_misc.tile([128, 512], f32, tag="misc", name="psvt")
                    for kt in range(4):
                        nc.tensor.matmul(
                            psvt[:, :cw],
                            r(wv_sb[:, kt, :]) if cw >= 256 else wv_sb[:, kt, :],
                            r(x2all[:, kt, ks]) if cw >= 256 else x2all[:, kt, ks],
                            start=(kt == 0),
                            stop=(kt == 3),
                        )
                    vt_sb = work.tile([128, 512], f32, tag="vt")
                    nc.vector.tensor_copy(vt_sb[:, :cw], psvt[:, :cw])
                    for j in range(cw // 128):
                        t = c * 4 + lo // 128 + j
                        psv = ps_misc.tile([128, 128], f32, tag="misc", name="psv")
                        nc.tensor.transpose(
                            psv, vt_sb[:, j * 128 : (j + 1) * 128], ident
                        )
                        o = t * 130
                        m1 = maskf_sb[:, t : t + 1]
                        nc.vector.tensor_scalar_mul(
                            vaug[:, o : o + 64], psv[:, 0:64], m1
                        )
                        nc.vector.tensor_copy(vaug[:, o + 64 : o + 65], m1)
                        nc.vector.tensor_scalar_mul(
                            vaug[:, o + 65 : o + 129], psv[:, 64:128], m1
                        )
                        nc.vector.tensor_copy(vaug[:, o + 129 : o + 130], m1)

                def emit_qproj(c, x1c=None):
                    if x1c is None:
                        x1c = xstream.tile([128, 4, 512], f32r, tag="xs", name="x1c")
                        nc.sync.dma_start(
                            out=x1c, in_=x1r[:, :, c * 512 : (c + 1) * 512]
                        )
                    psq = ps_misc.tile([128, 512], f32, tag="misc", name="psq")
                    for kt in range(4):
                        nc.tensor.matmul(
                            psq,
                            r(wq_sb[:, kt, :]),
                            r(x1c[:, kt, :]),
                            start=(kt == 0),
                            stop=(kt == 3),
                        )
                    nc.vector.tensor_copy(q_t[:, c * 512 : (c + 1) * 512], psq)

                def emit_av(oacc0, oacc1, t, pt0, pt1):
                    nc.tensor.matmul(
                        oacc0,
                        r(vaug[:, t * 130 : t * 130 + 65]),
                        r(pt0),
                        start=(t == 0),
                        stop=(t == NT - 1),
                    )
                    nc.tensor.matmul(
                        oacc1,
                        r(vaug[:, t * 130 + 65 : t * 130 + 130]),
                        r(pt1),
                        start=(t == 0),
                        stop=(t == NT - 1),
                    )

                def emit_norm_proj(c, oacc0, oacc1, split=False):
                    # normalize: rows 0..63 are sum(P*V), row 64 is sum(P*mask)
                    qs = slice(c * 512, (c + 1) * 512)
                    heads = []
                    for oacc, o_n in ((oacc0, o_n0), (oacc1, o_n1)):
                        recip = work.tile([65, 512], f32r, tag="recip")
                        nc.vector.reciprocal(recip[64:65, :], oacc[64:65, :])
                        rb_ps = ps_misc.tile([128, 512], f32, tag="misc", name="rb_ps")
                        nc.tensor.matmul(
                            rb_ps,
                            r(ones65[64:65, :]),
                            r(recip[64:65, :]),
                            start=True,
                            stop=True,
                        )
                        rb_sb = work.tile([128, 512], f32, tag="rb")
                        if split:
                            nc.scalar.activation(out=rb_sb, in_=rb_ps, func=CPY)
                        else:
                            nc.vector.tensor_copy(rb_sb, rb_ps)
                        heads.append((oacc, o_n, rb_sb))
                    if not split:
                        for oacc, o_n, rb_sb in heads:
                            nc.vector.tensor_mul(
                                o_n[:, qs], oacc[0:64, :], rb_sb[0:64, :]
                            )
                    # output projection for this chunk's 4 row tiles
                    for st in range(4 * c, 4 * (c + 1)):
                        ss = slice(st * 128, (st + 1) * 128)
                        if split:
                            j = st - 4 * c
                            js = slice(j * 128, (j + 1) * 128)
                            for oacc, o_n, rb_sb in heads:
                                nc.vector.tensor_mul(
                                    o_n[:, ss], oacc[0:64, js], rb_sb[0:64, js]
                                )
                        tp = ps_misc.tile([128, 512], f32, tag="misc", name="tp")
                        nc.tensor.matmul(
                            tp,
                            r(o_n0[:, ss]),
                            r(wo2_sb[:, 0:512]),
                            start=True,
                            stop=False,
                        )
                        nc.tensor.matmul(
                            tp,
                            r(o_n1[:, ss]),
                            r(wo2_sb[:, 512:1024]),
                            start=False,
                            stop=True,
                        )
                        out_sb = work.tile([128, 512], f32, tag="outsb", bufs=4)
                        if split:
                            nc.scalar.activation(out=out_sb, in_=tp, func=CPY)
                        else:
                            nc.vector.tensor_copy(out_sb, tp)
                        nc.sync.dma_start(out=out[ss, :], in_=out_sb)

                emit_qproj(0, x1c=x1c0 if _rep == 0 else None)
                # K projection for just the first key tile (128 cols) so the
                # first score matmul fires as soon as possible
                ksplit = min(128, skc)
                psk0 = ps_misc.tile([128, 128], f32, tag="misc", name="psk0")
                for kt in range(4):
                    nc.tensor.matmul(
                        psk0[:, :ksplit],
                        wk_sb[:, kt, :],
                        x2all[:, kt, 0:ksplit],
                        start=(kt == 0),
                        stop=(kt == 3),
                    )
                nc.vector.tensor_copy(k_t[:, 0:ksplit], psk0[:, :ksplit])

                def emit_scores_exp(c, t):
                    qs_c = slice(c * 512, (c + 1) * 512)
                    sc = ps_big.tile([128, 1024], f32, tag="sc", name="sc")
                    nc.tensor.matmul(
                        sc[:, 0:512],
                        r(k_t[0:64, t * 128 : (t + 1) * 128]),
                        r(q_t[0:64, qs_c]),
                        start=True,
                        stop=True,
                    )
                    nc.tensor.matmul(
                        sc[:, 512:1024],
                        r(k_t[64:128, t * 128 : (t + 1) * 128]),
                        r(q_t[64:128, qs_c]),
                        start=True,
                        stop=True,
                    )
                    pt = pexp.tile([128, 1024], f32r)
                    nc.scalar.activation(out=pt, in_=sc, func=EXP, scale=0.125)
                    return pt[:, 0:512], pt[:, 512:1024]

                prev_chunk = None  # (c, oacc0, oacc1) not yet normalized
                pending = []  # [(oacc0, oacc1, t, pt0, pt1)] w/o AV emitted yet
                pt_carry = None  # exp output for (c, t=0) computed in chunk c-1
                for c in range(NQC):
                    qs = slice(c * 512, (c + 1) * 512)
                    oacc0 = ps_oacc.tile([65, 512], f32, tag="oacc", name="oacc0")
                    oacc1 = ps_oacc.tile([65, 512], f32, tag="oacc", name="oacc1")

                    for t in range(NT):
                        if t == 0 and pt_carry is not None:
                            pt0, pt1 = pt_carry
                            pt_carry = None
                        else:
                            pt0, pt1 = emit_scores_exp(c, t)
                        # stream later key-chunk projections into chunk 0
                        if c == 0 and t == 0 and skc > ksplit:
                            emit_kv(0, lo=0, hi=min(512, skc))  # V + vaug 0..3
                        if c == 0 and t % 4 == 1 and (kc := t // 4 + 1) < NKC:
                            emit_kv(kc)
                        if t == min(7, NT - 1) and prev_chunk is not None:
                            # all of the previous chunk's AV matmuls must be
                            # emitted before its normalization reads oacc
                            while pending and pending[0][0] is prev_chunk[1]:
                                emit_av(*pending.pop(0))
                            emit_norm_proj(*prev_chunk)
                            prev_chunk = None
                        if t == NT // 2 and c + 1 < NQC:
                            emit_qproj(c + 1)
                        if t == NT - 1 and c + 1 < NQC:
                            pt_carry = emit_scores_exp(c + 1, 0)
                        pending.append((oacc0, oacc1, t, pt0, pt1))
                        # during chunk 0 the PE also streams K/V projections;
                        # letting AV lag deeper keeps scores (which gate the
                        # ScalarE exp stream) flowing
                        depth = 7 if c == 0 else 6
                        while len(pending) >= depth:
                            emit_av(*pending.pop(0))
                    prev_chunk = (c, oacc0, oacc1)
                while pending:
                    emit_av(*pending.pop(0))
                emit_norm_proj(*prev_chunk, split=True)

    nc.compile()
    return nc


def _get_runtime(skc: int, reps: int = 1):
    key = (skc, reps)
    if key not in _RUNTIMES:
        _RUNTIMES[key] = _build_program(skc, reps)
    return _RUNTIMES[key]


def _numpy_reference(x1, x2, mask, Wq, bq, Wk, bk, Wv, bv, Wo, bo):
    q = (x1 @ Wq + bq).reshape(B, S, H, DH).transpose(0, 2, 1, 3)
    k = (x2 @ Wk + bk).reshape(B, S, H, DH).transpose(0, 2, 1, 3)
    v = (x2 @ Wv + bv).reshape(B, S, H, DH).transpose(0, 2, 1, 3)
    scores = np.einsum("bhqd,bhkd->bhqk", q, k) / np.sqrt(np.float32(DH))
    scores = scores + mask[:, None, None, :].astype(np.float32) * np.float32(-1e9)
    scores = scores - scores.max(axis=-1, keepdims=True)
    e = np.exp(scores)
    attn = e / e.sum(axis=-1, keepdims=True)
    o = np.einsum("bhqk,bhkd->bhqd", attn, v)
    o = o.transpose(0, 2, 1, 3).reshape(B, S, D)
    return (o @ Wo + bo).astype(np.float32)


def _make_in_maps(x1, x2, mask, Wq, Wk, Wv, Wo):
    keep = [np.nonzero(mask[b] == 0)[0] for b in range(B)]
    counts = [len(k) for k in keep]
    skc = ((max(counts) + 127) // 128) * 128
    nt = skc // 128
    in_maps = []
    for c in range(NCORES):
        b, hp = c // 4, c % 4
        x2c = np.zeros((skc, D), dtype=np.float32)
        x2c[: counts[b]] = x2[b][keep[b]]
        mf = np.zeros((nt, 128), dtype=np.float32)
        mf.reshape(-1)[: counts[b]] = 1.0
        cols = slice(hp * 128, (hp + 1) * 128)
        wo2 = np.empty((64, 1024), dtype=np.float32)
        wo2[:, 0:512] = Wo[hp * 128 : hp * 128 + 64, :]
        wo2[:, 512:1024] = Wo[hp * 128 + 64 : (hp + 1) * 128, :]
        in_maps.append(
            {
                "x1t": np.ascontiguousarray(x1[b].T),
                "x2ct": np.ascontiguousarray(x2c.T),
                "maskf": np.ascontiguousarray(mf.T),
                "wq": np.ascontiguousarray(Wq[:, cols]),
                "wk": np.ascontiguousarray(Wk[:, cols]),
                "wv": np.ascontiguousarray(Wv[:, cols]),
                "wo2": wo2,
            }
        )
    return skc, in_maps


def kernel(x1, x2, mask, Wq, bq, Wk, bk, Wv, bv, Wo, bo):
    from concourse.bass_utils import run_bass_kernel_spmd

    x1 = np.asarray(x1, dtype=np.float32)
    x2 = np.asarray(x2, dtype=np.float32)
    mask = np.asarray(mask)
    Wq = np.asarray(Wq, dtype=np.float32)
    Wk = np.asarray(Wk, dtype=np.float32)
    Wv = np.asarray(Wv, dtype=np.float32)
    Wo = np.asarray(Wo, dtype=np.float32)
    bq, bk, bv, bo = (np.asarray(b, dtype=np.float32) for b in (bq, bk, bv, bo))

    counts = [int((mask[b] == 0).sum()) for b in range(B)]
    if any(np.abs(b).max() > 0 for b in (bq, bk, bv) if b.size) or min(counts) == 0:
        return _numpy_reference(x1, x2, mask, Wq, bq, Wk, bk, Wv, bv, Wo, bo)

    skc, in_maps = _make_in_maps(x1, x2, mask, Wq, Wk, Wv, Wo)
    nc = _get_runtime(skc)

    res = run_bass_kernel_spmd(nc, in_maps, core_ids=list(range(NCORES)))
    full = np.empty((B, S, D), dtype=np.float32)
    for b in range(B):
        acc = res.results[4 * b]["out"]
        for hp in range(1, 4):
            acc = acc + res.results[4 * b + hp]["out"]
        full[b] = acc + bo
    return full



# revision 15
# speedup vs baseline: 1.2507x; 1.2507x over previous
"""Trainium2 Bass kernel for MultiHeadAttention (B=2, S=4096, D=512, H=8).

Sharding: 16 (batch, head) units across 8 cores -> each core owns one batch
and a contiguous pair of heads (2 heads x 64 depth = 128 columns of the
QKV projections, 128 rows of the output projection).

Key ideas (v2 — bf16 + restructured PE work + 2-engine softmax exp):
  * Mask compression on host: keys with mask==1 receive -1e9 before softmax,
    so their probability is exactly 0 in fp32. We drop those keys entirely
    (gather unmasked rows of x2), roughly halving scores/softmax/AV work.
  * All matmuls run in bf16 (fp32 PSUM accumulation). The 1/8 score scale is
    folded into Wk on the host, the exp therefore needs no scale.
  * Scores land as [128 keys, 1024(=2 heads x 512 queries)] in PSUM.
  * The A@V matmuls use the probability tile as the STATIONARY operand
    ([128 keys, 128 queries] slices) and [V | mask] as the 65-column moving
    operand, producing [128 queries, 65] accumulators: 8x65 PE rows per key
    tile instead of 2x512 — half the PE time of the query-moving form. The
    65th column accumulates sum(p*mask) = the softmax denominator.
  * V is projected directly into key-major layout (x2 chunk stationary,
    Wv moving), so no V transpose pass is needed.
  * Normalization: reciprocal of the denominator column + per-partition
    tensor_scalar multiplies on DVE (queries live on partitions after AV).
  * The normalized output [128 q, 128 d] is transposed (PE, bf16) and fed to
    a packed output projection with full 128-deep contraction.
  * Softmax exp is split between the Scalar engine (activation Exp straight
    from PSUM) and the otherwise-idle GPSIMD engine (tensor_tensor pow:
    e ** scores, via an SBUF copy made by DVE), so no single engine owns the
    16.8M-element exponential.
  * Host sums the 4 per-core partial outputs of each batch (head groups are
    disjoint in Wo rows, so partials just add; bo added on host).

Non-zero q/k/v biases or an all-masked batch fall back to a numpy reference
(those inputs cannot occur with the problem's setup_inputs).
"""

import numpy as np

B, S, D, H = 2, 4096, 512, 8
DH = 64  # depth per head
NCORES = 8

_RUNTIMES = {}

_E = float(np.e)


def _build_program(skc: int, reps: int = 1):
    """Build the per-core Bass program. skc = padded compressed key count."""
    import concourse.bacc as bacc
    import concourse.mybir as mybir
    from concourse.masks import make_identity
    from concourse.tile import TileContext

    f32 = mybir.dt.float32
    bf16 = mybir.dt.bfloat16
    EXP = mybir.ActivationFunctionType.Exp
    CPY = mybir.ActivationFunctionType.Copy
    POW = mybir.AluOpType.pow

    NT = skc // 128  # key tiles
    NQC = S // 512  # query chunks (512 wide)
    NKC = (skc + 511) // 512  # key chunks for the K/V projections

    # exp-engine assignment: which (key tile, head) halves go to GPSIMD (pow,
    # via a DVE PSUM->SBUF copy) instead of ScalarE. Chunk 0's DVE is busy
    # with K/V copies, so fewer pool halves there.
    def pool_half(c, t, h):
        i = 2 * t + h
        if c == 0:
            return i % 4 == 3
        return i % 9 in (1, 3, 5, 7)

    nc = bacc.Bacc("TRN2", target_bir_lowering=False, debug=False, num_devices=NCORES)

    x1t = nc.dram_tensor("x1t", [D, S], bf16, kind="ExternalInput")
    x2ct = nc.dram_tensor("x2ct", [D, skc], bf16, kind="ExternalInput")
    maskf = nc.dram_tensor("maskf", [128, NT], f32, kind="ExternalInput")
    wq = nc.dram_tensor("wq", [D, 128], bf16, kind="ExternalInput")
    wk = nc.dram_tensor("wk", [D, 128], bf16, kind="ExternalInput")
    wv = nc.dram_tensor("wv", [D, 128], bf16, kind="ExternalInput")
    wo2 = nc.dram_tensor("wo2", [128, 512], bf16, kind="ExternalInput")
    out = nc.dram_tensor("out", [S, D], f32, kind="ExternalOutput")

    with nc.allow_low_precision(
        reason="bf16 matmuls accumulate in fp32 PSUM; tolerance is 2e-2"
    ), TileContext(nc) as tc:
        with (
            tc.tile_pool(name="consts", bufs=1) as consts,
            tc.tile_pool(name="bigsb", bufs=1) as bigsb,
            tc.tile_pool(name="xstream", bufs=3) as xstream,
            tc.tile_pool(name="pexp", bufs=26) as pexp,
            tc.tile_pool(name="scsb", bufs=5) as scsb,
            tc.tile_pool(name="work", bufs=2) as work,
            tc.tile_pool(name="ps_sc", bufs=4, space="PSUM") as ps_sc,
            tc.tile_pool(name="ps_av", bufs=1, space="PSUM") as ps_av,
            tc.tile_pool(name="ps_misc", bufs=2, space="PSUM") as ps_misc,
        ):
            # ---- constants / persistent buffers (DMA issue order matters) ----
            x1r = x1t.rearrange("(t p) s -> p t s", p=128)
            wq_sb = consts.tile([128, 4, 128], bf16)
            nc.sync.dma_start(out=wq_sb, in_=wq.rearrange("(t p) m -> p t m", p=128))
            x1c0 = xstream.tile([128, 4, 512], bf16, tag="xs")
            for kt in range(4):
                nc.sync.dma_start(out=x1c0[:, kt, :], in_=x1r[:, kt, 0:512])
            wk_sb = consts.tile([128, 4, 128], bf16)
            nc.sync.dma_start(out=wk_sb, in_=wk.rearrange("(t p) m -> p t m", p=128))
            x2all = bigsb.tile([128, 4, skc], bf16)
            x2r = x2ct.rearrange("(t p) s -> p t s", p=128)
            c0w = min(512, skc)
            c0a = min(128, c0w)  # first key-tile lands fast -> early first score
            nc.sync.dma_start(out=x2all[:, :, 0:c0a], in_=x2r[:, :, 0:c0a])
            wv_sb = consts.tile([128, 4, 128], bf16)
            nc.sync.dma_start(out=wv_sb, in_=wv.rearrange("(t p) m -> p t m", p=128))
            maskf_sb = consts.tile([128, NT], f32)
            nc.sync.dma_start(out=maskf_sb, in_=maskf[:, :])
            if c0w > c0a:
                nc.sync.dma_start(out=x2all[:, :, c0a:c0w], in_=x2r[:, :, c0a:c0w])
            for c in range(1, NKC):
                cw = min(512, skc - c * 512)
                nc.sync.dma_start(
                    out=x2all[:, :, c * 512 : c * 512 + cw],
                    in_=x2r[:, :, c * 512 : c * 512 + cw],
                )
            wo2_sb = consts.tile([128, 512], bf16)
            nc.sync.dma_start(out=wo2_sb, in_=wo2[:, :])

            ident_f = consts.tile([128, 128], f32)
            make_identity(nc, ident_f)
            ident_bf = consts.tile([128, 128], bf16)
            nc.vector.tensor_copy(ident_bf, ident_f)
            const_e = consts.tile([128, 1024], f32)
            nc.vector.memset(const_e, _E)
            mf_bf = consts.tile([128, NT], bf16)
            nc.vector.tensor_copy(mf_bf, maskf_sb)

            # ---- persistent activations ----
            q_t = bigsb.tile([128, S], bf16)
            k_t = bigsb.tile([128, skc], bf16)
            vaug = bigsb.tile([128, NT * 130], bf16)

            # mask columns of vaug (columns 64 and 129 of each 130-block):
            # zero for padded key rows, one for kept keys -> they carry the
            # softmax denominator through the same PE accumulation as A@V.
            # Written by the otherwise-idle GPSIMD engine.
            for t in range(NT):
                nc.gpsimd.tensor_copy(
                    vaug[:, t * 130 + 64 : t * 130 + 65], mf_bf[:, t : t + 1]
                )
                nc.gpsimd.tensor_copy(
                    vaug[:, t * 130 + 129 : t * 130 + 130], mf_bf[:, t : t + 1]
                )

            for _rep in range(reps):

                def emit_k(c, lo=0, hi=None):
                    """K^T projection for key-chunk c, columns [lo, hi)."""
                    cw = min(512, skc - c * 512) if hi is None else hi
                    ks = slice(c * 512 + lo, c * 512 + cw)
                    cw = cw - lo
                    psk = ps_misc.tile([128, 512], f32, tag="misc", name="psk")
                    for kt in range(4):
                        nc.tensor.matmul(
                            psk[:, :cw],
                            wk_sb[:, kt, :],
                            x2all[:, kt, ks],
                            start=(kt == 0),
                            stop=(kt == 3),
                        )
                    nc.vector.tensor_copy(k_t[:, ks], psk[:, :cw])

                def emit_v(t):
                    """Key-major V projection for key tile t straight into
                    vaug (x2 chunk stationary, Wv moving)."""
                    psv = ps_misc.tile([128, 128], f32, tag="misc", name="psv")
                    ks = slice(t * 128, (t + 1) * 128)
                    for j in range(4):
                        nc.tensor.matmul(
                            psv,
                            x2all[:, j, ks],
                            wv_sb[:, j, :],
                            start=(j == 0),
                            stop=(j == 3),
                        )
                    o = t * 130
                    dst = vaug[:, o : o + 130].rearrange("p (b x) -> p b x", b=2)
                    src = psv.rearrange("p (b x) -> p b x", b=2)
                    nc.vector.tensor_copy(dst[:, :, 0:64], src[:, :, 0:64])

                def fetch_x1(c):
                    x1c = xstream.tile([128, 4, 512], bf16, tag="xs", name="x1c")
                    nc.sync.dma_start(out=x1c, in_=x1r[:, :, c * 512 : (c + 1) * 512])
                    return x1c

                def emit_qproj(c, x1c):
                    psq = ps_misc.tile([128, 512], f32, tag="misc", name="psq")
                    for kt in range(4):
                        nc.tensor.matmul(
                            psq,
                            wq_sb[:, kt, :],
                            x1c[:, kt, :],
                            start=(kt == 0),
                            stop=(kt == 3),
                        )
                    nc.scalar.activation(
                        out=q_t[:, c * 512 : (c + 1) * 512], in_=psq, func=CPY
                    )

                def emit_scores_exp(c, t):
                    """Scores for (chunk c, key tile t), one [128, 512] PSUM
                    tile per head, exp'd on ScalarE or (DVE copy + GPSIMD
                    pow) per half."""
                    qs_c = slice(c * 512, (c + 1) * 512)
                    tcs = slice(t * 128, (t + 1) * 128)
                    pts = []
                    for h in range(2):
                        sc = ps_sc.tile([128, 512], f32, tag="sc", name="sc")
                        nc.tensor.matmul(
                            sc,
                            k_t[h * 64 : h * 64 + 64, tcs],
                            q_t[h * 64 : h * 64 + 64, qs_c],
                            start=True,
                            stop=True,
                        )
                        pt = pexp.tile([128, 512], bf16)
                        if pool_half(c, t, h):
                            scs = scsb.tile([128, 512], f32, tag="scs")
                            nc.vector.tensor_copy(scs, sc)
                            nc.gpsimd.tensor_tensor(
                                out=pt, in0=const_e[:, 0:512], in1=scs, op=POW
                            )
                        else:
                            nc.scalar.activation(out=pt, in_=sc, func=EXP)
                        pts.append(pt)
                    return pts

                def emit_av(av0, av1, t, pt0, pt1):
                    # start=True zeroes the whole 2KB PSUM bank, so only the
                    # first write into each av bank may set it; the other
                    # three slots accumulate onto the zeroed bank.
                    for j in range(4):
                        nc.tensor.matmul(
                            av0[:, j * 65 : (j + 1) * 65],
                            pt0[:, j * 128 : (j + 1) * 128],
                            vaug[:, t * 130 : t * 130 + 65],
                            start=(t == 0 and j == 0),
                            stop=(t == NT - 1),
                        )
                        nc.tensor.matmul(
                            av1[:, j * 65 : (j + 1) * 65],
                            pt1[:, j * 128 : (j + 1) * 128],
                            vaug[:, t * 130 + 65 : t * 130 + 130],
                            start=(t == 0 and j == 0),
                            stop=(t == NT - 1),
                        )

                class NormOut:
                    """Phased normalize + transpose + output projection for a
                    finished chunk, spread across the next chunk's tile loop
                    so no engine sees a burst."""

                    def __init__(self, c, av0, av1):
                        self.c, self.av0, self.av1 = c, av0, av1
                        self.rc = None
                        self.o_pk = None
                        self.tr = None
                        self.o_nt = None

                    def phase(self, p):
                        c, av0, av1 = self.c, self.av0, self.av1
                        if p == 0:
                            self.rc = work.tile([128, 8], f32, tag="rc")
                            nc.vector.reciprocal(
                                self.rc[:, 0:4], av0[:, 64 : 64 + 196 : 65]
                            )
                            nc.vector.reciprocal(
                                self.rc[:, 4:8], av1[:, 64 : 64 + 196 : 65]
                            )
                            self.o_pk = work.tile([128, 4, 128], bf16, tag="opk")
                        elif 1 <= p <= 4:
                            j = p - 1
                            nc.vector.tensor_scalar_mul(
                                self.o_pk[:, j, 0:64],
                                av0[:, j * 65 : j * 65 + 64],
                                self.rc[:, j : j + 1],
                            )
                            nc.vector.tensor_scalar_mul(
                                self.o_pk[:, j, 64:128],
                                av1[:, j * 65 : j * 65 + 64],
                                self.rc[:, j + 4 : j + 5],
                            )
                        elif p == 5:
                            self.tr = ps_misc.tile([128, 512], bf16, tag="misc", name="tr")
                            for j in range(4):
                                # same bank-zeroing rule as emit_av
                                nc.tensor.matmul(
                                    self.tr[:, j * 128 : (j + 1) * 128],
                                    self.o_pk[:, j, :],
                                    ident_bf,
                                    is_transpose=True,
                                    start=(j == 0),
                                    stop=True,
                                )
                        elif p == 6:
                            self.o_nt = work.tile([128, 512], bf16, tag="ont")
                            nc.vector.tensor_copy(self.o_nt, self.tr)
                        elif 7 <= p <= 10:
                            j = p - 7
                            st = 4 * c + j
                            ss = slice(st * 128, (st + 1) * 128)
                            tp = ps_misc.tile([128, 512], f32, tag="misc", name="tp")
                            nc.tensor.matmul(
                                tp,
                                self.o_nt[:, j * 128 : (j + 1) * 128],
                                wo2_sb,
                                start=True,
                                stop=True,
                            )
                            out_sb = work.tile([128, 512], f32, tag="outsb", bufs=4)
                            if j < 2:
                                nc.scalar.activation(out=out_sb, in_=tp, func=CPY)
                            else:
                                nc.vector.tensor_copy(out_sb, tp)
                            nc.sync.dma_start(out=out[ss, :], in_=out_sb)

                # tile index (within the next chunk) -> NormOut phase number
                phase_at = {2: 0, 3: 1, 4: 2, 5: 3, 6: 4, 8: 5, 9: 6,
                            10: 7, 11: 8, 12: 9, 13: 10}

                emit_qproj(0, x1c0 if _rep == 0 else fetch_x1(0))
                # K projection for just the first key tile so the first score
                # matmul fires as soon as possible
                ksplit = min(128, skc)
                emit_k(0, 0, ksplit)

                norm = None  # NormOut of the previous chunk
                pending = []  # [(av0, av1, t, pt)] without AV emitted yet
                x1next = None
                for c in range(NQC):
                    av0 = ps_av.tile([128, 260], f32, tag="av0", name="av0")
                    av1 = ps_av.tile([128, 260], f32, tag="av1", name="av1")
                    for t in range(NT):
                        pt0, pt1 = emit_scores_exp(c, t)
                        if c == 0:
                            if t == 0 and skc > ksplit:
                                emit_k(0, ksplit, c0w)
                            if t % 4 == 1 and (kc := t // 4 + 1) < NKC:
                                emit_k(kc)
                            emit_v(t)
                        if t == 1 and c + 1 < NQC:
                            x1next = fetch_x1(c + 1)
                        if norm is not None and t >= 2:
                            if t == 2:
                                # all of the previous chunk's AV matmuls must
                                # be emitted before its normalization reads av
                                while pending and pending[0][0] is norm.av0:
                                    emit_av(*pending.pop(0))
                            p = phase_at.get(t)
                            if p is not None:
                                norm.phase(p)
                                if p == 10:
                                    norm = None
                        if t == NT // 2 and c + 1 < NQC:
                            emit_qproj(c + 1, x1next)
                        pending.append((av0, av1, t, pt0, pt1))
                        depth = 7 if c == 0 else (4 if c == NQC - 1 else 11)
                        while len(pending) >= depth:
                            emit_av(*pending.pop(0))
                    norm = NormOut(c, av0, av1)
                while pending:
                    emit_av(*pending.pop(0))
                for p in range(11):
                    norm.phase(p)

    nc.compile()
    return nc


def _get_runtime(skc: int, reps: int = 1):
    key = (skc, reps)
    if key not in _RUNTIMES:
        _RUNTIMES[key] = _build_program(skc, reps)
    return _RUNTIMES[key]


def _numpy_reference(x1, x2, mask, Wq, bq, Wk, bk, Wv, bv, Wo, bo):
    q = (x1 @ Wq + bq).reshape(B, S, H, DH).transpose(0, 2, 1, 3)
    k = (x2 @ Wk + bk).reshape(B, S, H, DH).transpose(0, 2, 1, 3)
    v = (x2 @ Wv + bv).reshape(B, S, H, DH).transpose(0, 2, 1, 3)
    scores = np.einsum("bhqd,bhkd->bhqk", q, k) / np.sqrt(np.float32(DH))
    scores = scores + mask[:, None, None, :].astype(np.float32) * np.float32(-1e9)
    scores = scores - scores.max(axis=-1, keepdims=True)
    e = np.exp(scores)
    attn = e / e.sum(axis=-1, keepdims=True)
    o = np.einsum("bhqk,bhkd->bhqd", attn, v)
    o = o.transpose(0, 2, 1, 3).reshape(B, S, D)
    return (o @ Wo + bo).astype(np.float32)


def _make_in_maps(x1, x2, mask, Wq, Wk, Wv, Wo):
    import ml_dtypes

    bf = ml_dtypes.bfloat16
    keep = [np.nonzero(mask[b] == 0)[0] for b in range(B)]
    counts = [len(k) for k in keep]
    skc = ((max(counts) + 127) // 128) * 128
    nt = skc // 128
    in_maps = []
    for c in range(NCORES):
        b, hp = c // 4, c % 4
        x2c = np.zeros((skc, D), dtype=np.float32)
        x2c[: counts[b]] = x2[b][keep[b]]
        mf = np.zeros((nt, 128), dtype=np.float32)
        mf.reshape(-1)[: counts[b]] = 1.0
        cols = slice(hp * 128, (hp + 1) * 128)
        wo2 = np.empty((128, 512), dtype=np.float32)
        wo2[0:64, :] = Wo[hp * 128 : hp * 128 + 64, :]
        wo2[64:128, :] = Wo[hp * 128 + 64 : (hp + 1) * 128, :]
        in_maps.append(
            {
                "x1t": np.ascontiguousarray(x1[b].T).astype(bf),
                "x2ct": np.ascontiguousarray(x2c.T).astype(bf),
                "maskf": np.ascontiguousarray(mf.T),
                "wq": np.ascontiguousarray(Wq[:, cols]).astype(bf),
                "wk": np.ascontiguousarray(Wk[:, cols] * np.float32(0.125)).astype(bf),
                "wv": np.ascontiguousarray(Wv[:, cols]).astype(bf),
                "wo2": wo2.astype(bf),
            }
        )
    return skc, in_maps


def kernel(x1, x2, mask, Wq, bq, Wk, bk, Wv, bv, Wo, bo):
    from concourse.bass_utils import run_bass_kernel_spmd

    x1 = np.asarray(x1, dtype=np.float32)
    x2 = np.asarray(x2, dtype=np.float32)
    mask = np.asarray(mask)
    Wq = np.asarray(Wq, dtype=np.float32)
    Wk = np.asarray(Wk, dtype=np.float32)
    Wv = np.asarray(Wv, dtype=np.float32)
    Wo = np.asarray(Wo, dtype=np.float32)
    bq, bk, bv, bo = (np.asarray(b, dtype=np.float32) for b in (bq, bk, bv, bo))

    counts = [int((mask[b] == 0).sum()) for b in range(B)]
    if any(np.abs(b).max() > 0 for b in (bq, bk, bv) if b.size) or min(counts) == 0:
        return _numpy_reference(x1, x2, mask, Wq, bq, Wk, bk, Wv, bv, Wo, bo)

    skc, in_maps = _make_in_maps(x1, x2, mask, Wq, Wk, Wv, Wo)
    nc = _get_runtime(skc)

    res = run_bass_kernel_spmd(nc, in_maps, core_ids=list(range(NCORES)))
    full = np.empty((B, S, D), dtype=np.float32)
    for b in range(B):
        acc = res.results[4 * b]["out"]
        for hp in range(1, 4):
            acc = acc + res.results[4 * b + hp]["out"]
        full[b] = acc + bo
    return full


# revision 32
# speedup vs baseline: 1.2571x; 1.0051x over previous
"""Trainium2 Bass kernel for MultiHeadAttention (B=2, S=4096, D=512, H=8).

Sharding: 16 (batch, head) units across 8 cores -> each core owns one batch
and a contiguous pair of heads (2 heads x 64 depth = 128 columns of the
QKV projections, 128 rows of the output projection).

Key ideas (v2 — bf16 + restructured PE work + 2-engine softmax exp):
  * Mask compression on host: keys with mask==1 receive -1e9 before softmax,
    so their probability is exactly 0 in fp32. We drop those keys entirely
    (gather unmasked rows of x2), roughly halving scores/softmax/AV work.
  * All matmuls run in bf16 (fp32 PSUM accumulation). The 1/8 score scale is
    folded into Wk on the host, the exp therefore needs no scale.
  * Scores land as [128 keys, 1024(=2 heads x 512 queries)] in PSUM.
  * The A@V matmuls use the probability tile as the STATIONARY operand
    ([128 keys, 128 queries] slices) and [V | mask] as the 65-column moving
    operand, producing [128 queries, 65] accumulators: 8x65 PE rows per key
    tile instead of 2x512 — half the PE time of the query-moving form. The
    65th column accumulates sum(p*mask) = the softmax denominator.
  * V is projected directly into key-major layout (x2 chunk stationary,
    Wv moving), so no V transpose pass is needed.
  * Normalization: reciprocal of the denominator column + per-partition
    tensor_scalar multiplies on DVE (queries live on partitions after AV).
  * The normalized output [128 q, 128 d] is transposed (PE, bf16) and fed to
    a packed output projection with full 128-deep contraction.
  * Softmax exp is split between the Scalar engine (activation Exp straight
    from PSUM) and the otherwise-idle GPSIMD engine (tensor_tensor pow:
    e ** scores, via an SBUF copy made by DVE), so no single engine owns the
    16.8M-element exponential.
  * Host sums the 4 per-core partial outputs of each batch (head groups are
    disjoint in Wo rows, so partials just add; bo added on host).

Non-zero q/k/v biases or an all-masked batch fall back to a numpy reference
(those inputs cannot occur with the problem's setup_inputs).
"""

import numpy as np

B, S, D, H = 2, 4096, 512, 8
DH = 64  # depth per head
NCORES = 8

_RUNTIMES = {}

_E = float(np.e)


def _build_program(skc: int, reps: int = 1):
    """Build the per-core Bass program. skc = padded compressed key count."""
    import concourse.bacc as bacc
    import concourse.mybir as mybir
    from concourse.masks import make_identity
    from concourse.tile import TileContext

    f32 = mybir.dt.float32
    bf16 = mybir.dt.bfloat16
    EXP = mybir.ActivationFunctionType.Exp
    CPY = mybir.ActivationFunctionType.Copy
    POW = mybir.AluOpType.pow

    NT = skc // 128  # key tiles
    NQC = S // 512  # query chunks (512 wide)
    NKC = (skc + 511) // 512  # key chunks for the K/V projections

    # exp-engine assignment: which (key tile, head) halves go to GPSIMD (pow,
    # via a DVE PSUM->SBUF copy) instead of ScalarE. Chunk 0's DVE is busy
    # with K/V copies, so fewer pool halves there.
    def pool_half(c, t, h):
        i = 2 * t + h
        if c == 0:
            return i % 4 == 3
        return i % 9 in (1, 3, 5, 7)

    nc = bacc.Bacc("TRN2", target_bir_lowering=False, debug=False, num_devices=NCORES)

    x1t = nc.dram_tensor("x1t", [D, S], bf16, kind="ExternalInput")
    x2ct = nc.dram_tensor("x2ct", [D, skc], bf16, kind="ExternalInput")
    maskf = nc.dram_tensor("maskf", [128, NT], f32, kind="ExternalInput")
    wqkv = nc.dram_tensor("wqkv", [D, 384], bf16, kind="ExternalInput")
    wo2 = nc.dram_tensor("wo2", [128, 512], bf16, kind="ExternalInput")
    out = nc.dram_tensor("out", [S, D], f32, kind="ExternalOutput")

    with nc.allow_low_precision(
        reason="bf16 matmuls accumulate in fp32 PSUM; tolerance is 2e-2"
    ), TileContext(nc) as tc:
        with (
            tc.tile_pool(name="consts", bufs=1) as consts,
            tc.tile_pool(name="bigsb", bufs=1) as bigsb,
            tc.tile_pool(name="xstream", bufs=3) as xstream,
            tc.tile_pool(name="pexp", bufs=26) as pexp,
            tc.tile_pool(name="scsb", bufs=5) as scsb,
            tc.tile_pool(name="work", bufs=2) as work,
            tc.tile_pool(name="ps_sc", bufs=4, space="PSUM") as ps_sc,
            tc.tile_pool(name="ps_av", bufs=1, space="PSUM") as ps_av,
            tc.tile_pool(name="ps_misc", bufs=2, space="PSUM") as ps_misc,
        ):
            # ---- constants / persistent buffers. Every dma_start costs
            # ~625ns of serialized HWDGE descriptor-gen, so the warmup path
            # uses few, large transfers: packed W_qkv first, then a small
            # lead slice of x2 (first key tile), then the x1 chunk. ----
            x1r = x1t.rearrange("(t p) s -> p t s", p=128)
            wqkv_sb = consts.tile([128, 4, 384], bf16)
            nc.sync.dma_start(
                out=wqkv_sb, in_=wqkv.rearrange("(t p) m -> p t m", p=128)
            )
            wq_sb = wqkv_sb[:, :, 0:128]
            wk_sb = wqkv_sb[:, :, 128:256]
            wv_sb = wqkv_sb[:, :, 256:384]
            x2all = bigsb.tile([128, 4, skc], bf16)
            x2r = x2ct.rearrange("(t p) s -> p t s", p=128)
            c0w = min(512, skc)
            c0a = min(128, c0w)  # first key-tile lands fast -> early first score
            nc.sync.dma_start(out=x2all[:, :, 0:c0a], in_=x2r[:, :, 0:c0a])
            x1c0 = xstream.tile([128, 4, 512], bf16, tag="xs")
            nc.sync.dma_start(out=x1c0, in_=x1r[:, :, 0:512])
            maskf_sb = consts.tile([128, NT], f32)
            nc.sync.dma_start(out=maskf_sb, in_=maskf[:, :])
            if c0w > c0a:
                nc.sync.dma_start(out=x2all[:, :, c0a:c0w], in_=x2r[:, :, c0a:c0w])
            x1c1 = None
            for c in range(1, NKC):
                cw = min(512, skc - c * 512)
                nc.sync.dma_start(
                    out=x2all[:, :, c * 512 : c * 512 + cw],
                    in_=x2r[:, :, c * 512 : c * 512 + cw],
                )
                if c == 1 and NQC > 1:
                    # prefetch chunk 1's x1 ahead of the later x2 chunks: it
                    # is needed earlier than they are
                    x1c1 = xstream.tile([128, 4, 512], bf16, tag="xs", name="x1c")
                    nc.sync.dma_start(out=x1c1, in_=x1r[:, :, 512:1024])
            wo2_sb = consts.tile([128, 512], bf16)
            nc.sync.dma_start(out=wo2_sb, in_=wo2[:, :])

            ident_f = consts.tile([128, 128], f32)
            make_identity(nc, ident_f)
            ident_bf = consts.tile([128, 128], bf16)
            nc.vector.tensor_copy(ident_bf, ident_f)
            const_e = consts.tile([128, 1024], f32)
            nc.vector.memset(const_e, _E)
            mf_bf = consts.tile([128, NT], bf16)
            nc.vector.tensor_copy(mf_bf, maskf_sb)

            # ---- persistent activations ----
            q_t = bigsb.tile([128, S], bf16)
            k_t = bigsb.tile([128, skc], bf16)
            vaug = bigsb.tile([128, NT * 130], bf16)

            # mask columns of vaug (columns 64 and 129 of each 130-block):
            # zero for padded key rows, one for kept keys -> they carry the
            # softmax denominator through the same PE accumulation as A@V.
            # Written by the otherwise-idle GPSIMD engine.
            for t in range(NT):
                nc.gpsimd.tensor_copy(
                    vaug[:, t * 130 + 64 : t * 130 + 65], mf_bf[:, t : t + 1]
                )
                nc.gpsimd.tensor_copy(
                    vaug[:, t * 130 + 129 : t * 130 + 130], mf_bf[:, t : t + 1]
                )

            for _rep in range(reps):

                def emit_k(c, lo=0, hi=None):
                    """K^T projection for key-chunk c, columns [lo, hi)."""
                    cw = min(512, skc - c * 512) if hi is None else hi
                    ks = slice(c * 512 + lo, c * 512 + cw)
                    cw = cw - lo
                    psk = ps_misc.tile([128, 512], f32, tag="misc", name="psk")
                    for kt in range(4):
                        nc.tensor.matmul(
                            psk[:, :cw],
                            wk_sb[:, kt, :],
                            x2all[:, kt, ks],
                            start=(kt == 0),
                            stop=(kt == 3),
                        )
                    nc.vector.tensor_copy(k_t[:, ks], psk[:, :cw])

                def emit_v(t):
                    """Key-major V projection for key tile t straight into
                    vaug (x2 chunk stationary, Wv moving)."""
                    psv = ps_misc.tile([128, 128], f32, tag="misc", name="psv")
                    ks = slice(t * 128, (t + 1) * 128)
                    for j in range(4):
                        nc.tensor.matmul(
                            psv,
                            x2all[:, j, ks],
                            wv_sb[:, j, :],
                            start=(j == 0),
                            stop=(j == 3),
                        )
                    o = t * 130
                    dst = vaug[:, o : o + 130].rearrange("p (b x) -> p b x", b=2)
                    src = psv.rearrange("p (b x) -> p b x", b=2)
                    nc.vector.tensor_copy(dst[:, :, 0:64], src[:, :, 0:64])

                def fetch_x1(c):
                    x1c = xstream.tile([128, 4, 512], bf16, tag="xs", name="x1c")
                    nc.sync.dma_start(out=x1c, in_=x1r[:, :, c * 512 : (c + 1) * 512])
                    return x1c

                def emit_qproj(c, x1c):
                    psq = ps_misc.tile([128, 512], f32, tag="misc", name="psq")
                    for kt in range(4):
                        nc.tensor.matmul(
                            psq,
                            wq_sb[:, kt, :],
                            x1c[:, kt, :],
                            start=(kt == 0),
                            stop=(kt == 3),
                        )
                    nc.scalar.activation(
                        out=q_t[:, c * 512 : (c + 1) * 512], in_=psq, func=CPY
                    )

                def emit_scores_exp(c, t):
                    """Scores for (chunk c, key tile t), one [128, 512] PSUM
                    tile per head, exp'd on ScalarE or (DVE copy + GPSIMD
                    pow) per half."""
                    qs_c = slice(c * 512, (c + 1) * 512)
                    tcs = slice(t * 128, (t + 1) * 128)
                    pts = []
                    for h in range(2):
                        sc = ps_sc.tile([128, 512], f32, tag="sc", name="sc")
                        nc.tensor.matmul(
                            sc,
                            k_t[h * 64 : h * 64 + 64, tcs],
                            q_t[h * 64 : h * 64 + 64, qs_c],
                            start=True,
                            stop=True,
                        )
                        pt = pexp.tile([128, 512], bf16)
                        if pool_half(c, t, h):
                            scs = scsb.tile([128, 512], f32, tag="scs")
                            nc.vector.tensor_copy(scs, sc)
                            nc.gpsimd.tensor_tensor(
                                out=pt, in0=const_e[:, 0:512], in1=scs, op=POW
                            )
                        else:
                            nc.scalar.activation(out=pt, in_=sc, func=EXP)
                        pts.append(pt)
                    return pts

                def emit_av(av0, av1, t, pt0, pt1):
                    # start=True zeroes the whole 2KB PSUM bank, so only the
                    # first write into each av bank may set it; the other
                    # three slots accumulate onto the zeroed bank.
                    for j in range(4):
                        nc.tensor.matmul(
                            av0[:, j * 65 : (j + 1) * 65],
                            pt0[:, j * 128 : (j + 1) * 128],
                            vaug[:, t * 130 : t * 130 + 65],
                            start=(t == 0 and j == 0),
                            stop=(t == NT - 1),
                        )
                        nc.tensor.matmul(
                            av1[:, j * 65 : (j + 1) * 65],
                            pt1[:, j * 128 : (j + 1) * 128],
                            vaug[:, t * 130 + 65 : t * 130 + 130],
                            start=(t == 0 and j == 0),
                            stop=(t == NT - 1),
                        )

                class NormOut:
                    """Phased normalize + transpose + output projection for a
                    finished chunk, spread across the next chunk's tile loop
                    so no engine sees a burst."""

                    def __init__(self, c, av0, av1, tail=False):
                        self.c, self.av0, self.av1 = c, av0, av1
                        self.tail = tail
                        self.rc = None
                        self.o_pk = None
                        self.tr = None
                        self.o_nt = None

                    def phase(self, p):
                        c, av0, av1 = self.c, self.av0, self.av1
                        if p == 0:
                            self.rc = work.tile([128, 8], f32, tag="rc")
                            nc.vector.reciprocal(
                                self.rc[:, 0:4], av0[:, 64 : 64 + 196 : 65]
                            )
                            nc.vector.reciprocal(
                                self.rc[:, 4:8], av1[:, 64 : 64 + 196 : 65]
                            )
                            self.o_pk = work.tile([128, 4, 128], bf16, tag="opk")
                        elif 1 <= p <= 4:
                            j = p - 1
                            nc.vector.tensor_scalar_mul(
                                self.o_pk[:, j, 0:64],
                                av0[:, j * 65 : j * 65 + 64],
                                self.rc[:, j : j + 1],
                            )
                            if self.tail:
                                # ScalarE is idle in the tail: normalize the
                                # h1 half there via a scaled copy
                                nc.scalar.activation(
                                    out=self.o_pk[:, j, 64:128],
                                    in_=av1[:, j * 65 : j * 65 + 64],
                                    func=CPY,
                                    scale=self.rc[:, j + 4 : j + 5],
                                )
                            else:
                                nc.vector.tensor_scalar_mul(
                                    self.o_pk[:, j, 64:128],
                                    av1[:, j * 65 : j * 65 + 64],
                                    self.rc[:, j + 4 : j + 5],
                                )
                        elif p == 5:
                            self.tr = ps_misc.tile([128, 512], bf16, tag="misc", name="tr")
                            for j in range(4):
                                # same bank-zeroing rule as emit_av
                                nc.tensor.matmul(
                                    self.tr[:, j * 128 : (j + 1) * 128],
                                    self.o_pk[:, j, :],
                                    ident_bf,
                                    is_transpose=True,
                                    start=(j == 0),
                                    stop=True,
                                )
                        elif p == 6:
                            self.o_nt = work.tile([128, 512], bf16, tag="ont")
                            nc.vector.tensor_copy(self.o_nt, self.tr)
                        elif 7 <= p <= 10:
                            j = p - 7
                            st = 4 * c + j
                            ss = slice(st * 128, (st + 1) * 128)
                            tp = ps_misc.tile([128, 512], f32, tag="misc", name="tp")
                            nc.tensor.matmul(
                                tp,
                                self.o_nt[:, j * 128 : (j + 1) * 128],
                                wo2_sb,
                                start=True,
                                stop=True,
                            )
                            out_sb = work.tile([128, 512], f32, tag="outsb", bufs=4)
                            sc_copy = (j % 2 == 0) if self.tail else (j < 2)
                            if sc_copy:
                                nc.scalar.activation(out=out_sb, in_=tp, func=CPY)
                            else:
                                nc.vector.tensor_copy(out_sb, tp)
                            nc.sync.dma_start(out=out[ss, :], in_=out_sb)

                # tile index (within the next chunk) -> NormOut phase number
                phase_at = {2: 0, 3: 1, 4: 2, 5: 3, 6: 4, 8: 5, 9: 6,
                            10: 7, 11: 8, 12: 9, 13: 10}

                emit_qproj(0, x1c0 if _rep == 0 else fetch_x1(0))
                # K projection for just the first key tile so the first score
                # matmul fires as soon as possible
                ksplit = min(128, skc)
                emit_k(0, 0, ksplit)

                norm = None  # NormOut of the previous chunk
                pending = []  # [(av0, av1, t, pt)] without AV emitted yet
                x1next = None
                for c in range(NQC):
                    av0 = ps_av.tile([128, 260], f32, tag="av0", name="av0")
                    av1 = ps_av.tile([128, 260], f32, tag="av1", name="av1")
                    for t in range(NT):
                        pt0, pt1 = emit_scores_exp(c, t)
                        if c == 0:
                            if t == 0 and skc > ksplit:
                                emit_k(0, ksplit, c0w)
                            if t in (3, 6, 9) and (kc := t // 3) < NKC:
                                emit_k(kc)
                            emit_v(t)
                        if t == 1 and c + 1 < NQC:
                            x1next = x1c1 if (c == 0 and x1c1 is not None and _rep == 0) else fetch_x1(c + 1)
                        if norm is not None and t >= 2:
                            if t == 2:
                                # all of the previous chunk's AV matmuls must
                                # be emitted before its normalization reads av
                                while pending and pending[0][0] is norm.av0:
                                    emit_av(*pending.pop(0))
                            p = phase_at.get(t)
                            if p is not None:
                                norm.phase(p)
                                if p == 10:
                                    norm = None
                        if t == 5 and c + 1 < NQC:
                            emit_qproj(c + 1, x1next)
                        pending.append((av0, av1, t, pt0, pt1))
                        depth = 7 if c == 0 else 15
                        while len(pending) >= depth:
                            emit_av(*pending.pop(0))
                    norm = NormOut(c, av0, av1)
                while pending:
                    emit_av(*pending.pop(0))
                for p in range(11):
                    norm.phase(p)

    nc.compile()
    return nc


def _get_runtime(skc: int, reps: int = 1):
    key = (skc, reps)
    if key not in _RUNTIMES:
        _RUNTIMES[key] = _build_program(skc, reps)
    return _RUNTIMES[key]


def _numpy_reference(x1, x2, mask, Wq, bq, Wk, bk, Wv, bv, Wo, bo):
    q = (x1 @ Wq + bq).reshape(B, S, H, DH).transpose(0, 2, 1, 3)
    k = (x2 @ Wk + bk).reshape(B, S, H, DH).transpose(0, 2, 1, 3)
    v = (x2 @ Wv + bv).reshape(B, S, H, DH).transpose(0, 2, 1, 3)
    scores = np.einsum("bhqd,bhkd->bhqk", q, k) / np.sqrt(np.float32(DH))
    scores = scores + mask[:, None, None, :].astype(np.float32) * np.float32(-1e9)
    scores = scores - scores.max(axis=-1, keepdims=True)
    e = np.exp(scores)
    attn = e / e.sum(axis=-1, keepdims=True)
    o = np.einsum("bhqk,bhkd->bhqd", attn, v)
    o = o.transpose(0, 2, 1, 3).reshape(B, S, D)
    return (o @ Wo + bo).astype(np.float32)


def _make_in_maps(x1, x2, mask, Wq, Wk, Wv, Wo):
    import ml_dtypes

    bf = ml_dtypes.bfloat16
    keep = [np.nonzero(mask[b] == 0)[0] for b in range(B)]
    counts = [len(k) for k in keep]
    skc = ((max(counts) + 127) // 128) * 128
    nt = skc // 128
    in_maps = []
    for c in range(NCORES):
        b, hp = c // 4, c % 4
        x2c = np.zeros((skc, D), dtype=np.float32)
        x2c[: counts[b]] = x2[b][keep[b]]
        mf = np.zeros((nt, 128), dtype=np.float32)
        mf.reshape(-1)[: counts[b]] = 1.0
        cols = slice(hp * 128, (hp + 1) * 128)
        wo2 = np.empty((128, 512), dtype=np.float32)
        wo2[0:64, :] = Wo[hp * 128 : hp * 128 + 64, :]
        wo2[64:128, :] = Wo[hp * 128 + 64 : (hp + 1) * 128, :]
        in_maps.append(
            {
                "x1t": np.ascontiguousarray(x1[b].T).astype(bf),
                "x2ct": np.ascontiguousarray(x2c.T).astype(bf),
                "maskf": np.ascontiguousarray(mf.T),
                "wqkv": np.ascontiguousarray(
                    np.concatenate(
                        [
                            Wq[:, cols],
                            Wk[:, cols] * np.float32(0.125),
                            Wv[:, cols],
                        ],
                        axis=1,
                    )
                ).astype(bf),
                "wo2": wo2.astype(bf),
            }
        )
    return skc, in_maps


def kernel(x1, x2, mask, Wq, bq, Wk, bk, Wv, bv, Wo, bo):
    from concourse.bass_utils import run_bass_kernel_spmd

    x1 = np.asarray(x1, dtype=np.float32)
    x2 = np.asarray(x2, dtype=np.float32)
    mask = np.asarray(mask)
    Wq = np.asarray(Wq, dtype=np.float32)
    Wk = np.asarray(Wk, dtype=np.float32)
    Wv = np.asarray(Wv, dtype=np.float32)
    Wo = np.asarray(Wo, dtype=np.float32)
    bq, bk, bv, bo = (np.asarray(b, dtype=np.float32) for b in (bq, bk, bv, bo))

    counts = [int((mask[b] == 0).sum()) for b in range(B)]
    if any(np.abs(b).max() > 0 for b in (bq, bk, bv) if b.size) or min(counts) == 0:
        return _numpy_reference(x1, x2, mask, Wq, bq, Wk, bk, Wv, bv, Wo, bo)

    skc, in_maps = _make_in_maps(x1, x2, mask, Wq, Wk, Wv, Wo)
    nc = _get_runtime(skc)

    res = run_bass_kernel_spmd(nc, in_maps, core_ids=list(range(NCORES)))
    full = np.empty((B, S, D), dtype=np.float32)
    for b in range(B):
        acc = res.results[4 * b]["out"]
        for hp in range(1, 4):
            acc = acc + res.results[4 * b + hp]["out"]
        full[b] = acc + bo
    return full
